# revision 1
# baseline (speedup 1.0000x reference)
"""Trainium2 Bass kernel for nn_AttentionBlock (GroupNorm32 + QKVAttentionLegacy + proj).

Sharding: 8 cores = 4 batch x 2 L-halves. Each core computes the full block for
one batch element restricted to a 2048-query half; keys/values span all 4096
positions. Odd-half cores receive x with the two L-halves swapped (attention is
permutation-invariant over key positions), so one SPMD program serves all cores
with static slicing and zero collectives.

Per-core algorithm (fp32 throughout):
  - GroupNorm folded into qkv weights: qkv = (W*A) @ x + (W@B + qkv_b), where
    A/B are the per-channel affine from group stats (computed on device).
  - scores^T = k_h^T q_h per head, computed 2-heads-at-a-time with row-tiled
    (64x128) matmuls; softmax without max-subtraction (logits are O(3));
    exp on ScalarE drains score PSUM directly.
  - a_un/den via one matmul per (s-tile, head): lhsT = [v^T | ones] (65 cols),
    accumulating [65, 512] in PSUM; den reciprocal batched via a DRAM bounce;
    normalization on VectorE.
  - v-bias and GroupNorm bias-through-v fold into the proj bias.
"""

import sys

import numpy as np

for _p in ("/opt/trn_rl_repo",):
    if _p not in sys.path:
        sys.path.insert(0, _p)

NUM_HEADS = 4
C = 256
L = 4096
T = 2048  # per-core query half
CH = 64
GROUPS = 32
EPS = 1e-5

_CACHE = {}


def _build_bass(loop_n=None):
    import concourse.tile as tile
    from concourse import bacc, mybir

    f32 = mybir.dt.float32
    AF = mybir.ActivationFunctionType
    OP = mybir.AluOpType

    nc = bacc.Bacc()
    f32r = mybir.dt.float32r
    R = lambda ap: ap.bitcast(f32r)  # FP22 matmul path: 1 cyc/row vs fp32's 4

    xp_d = nc.dram_tensor("xp", [C, L], f32, kind="ExternalInput")
    wqkvT_d = nc.dram_tensor("wqkvT", [C, 3 * C], f32, kind="ExternalInput")
    wprojT_d = nc.dram_tensor("wprojT", [C, C], f32, kind="ExternalInput")
    qkvb_d = nc.dram_tensor("qkvb", [3 * C], f32, kind="ExternalInput")
    gnw_d = nc.dram_tensor("gnw", [C], f32, kind="ExternalInput")
    gnb_d = nc.dram_tensor("gnb", [C], f32, kind="ExternalInput")
    projb_d = nc.dram_tensor("projb", [C], f32, kind="ExternalInput")
    out_d = nc.dram_tensor("out", [C, T], f32, kind="ExternalOutput")

    # group indicator matrices for partition-dim group reductions via PE
    ig_np = np.zeros((C, GROUPS), np.float32)
    ig_np[np.arange(C), np.arange(C) // 8] = 1.0
    ig_d = nc.inline_tensor(ig_np, "ig")
    igT_d = nc.inline_tensor(np.ascontiguousarray(ig_np.T), "igT")

    with tile.TileContext(nc) as tc:
        from contextlib import ExitStack, nullcontext

        ctx = ExitStack()
        with ctx:
            loop = tc.For_i(0, loop_n, 1) if loop_n else nullcontext()
            ctx.enter_context(loop)
            singles = ctx.enter_context(tc.tile_pool(name="singles", bufs=1))
            ew_pool = ctx.enter_context(tc.tile_pool(name="ew", bufs=4))
            small = ctx.enter_context(tc.tile_pool(name="small", bufs=2))
            outp = ctx.enter_context(tc.tile_pool(name="outp", bufs=2))
            dram = ctx.enter_context(
                tc.tile_pool(name="dramp", bufs=4, space="DRAM")
            )

            # ---------------- load inputs ----------------
            xs = singles.tile([128, 2, L], f32, tag="xs")
            for t in range(2):
                for j in range(2):
                    nc.sync.dma_start(
                        out=R(xs[:, t, j * 2048 : (j + 1) * 2048]),
                        in_=xp_d[t * 128 : (t + 1) * 128, j * 2048 : (j + 1) * 2048].bitcast(f32r),
                    )
            pt4 = singles.tile([64, 4, C], f32, tag="pt4")
            for h in range(4):
                nc.sync.dma_start(
                    out=R(pt4[:, h, :]), in_=wprojT_d[h * 64 : (h + 1) * 64, :].bitcast(f32r)
                )
            qkvb = singles.tile([128, 6], f32, tag="qkvb")
            nc.sync.dma_start(out=qkvb, in_=qkvb_d[:].rearrange("(m p) -> p m", p=128))
            gnw = singles.tile([128, 2], f32, tag="gnw")
            nc.sync.dma_start(out=gnw, in_=gnw_d[:].rearrange("(t p) -> p t", p=128))
            gnb = singles.tile([128, 2], f32, tag="gnb")
            nc.sync.dma_start(out=gnb, in_=gnb_d[:].rearrange("(t p) -> p t", p=128))
            projb = singles.tile([128, 2], f32, tag="projb")
            nc.sync.dma_start(out=projb, in_=projb_d[:].rearrange("(t p) -> p t", p=128))
            ig = singles.tile([128, 2, GROUPS], f32, tag="ig")
            for t in range(2):
                nc.sync.dma_start(out=ig[:, t, :], in_=ig_d[t * 128 : (t + 1) * 128, :])
            igT = singles.tile([GROUPS, C], f32, tag="igT")
            nc.sync.dma_start(out=igT, in_=igT_d[:, :])

            # ---------------- GroupNorm stats -> A, B ----------------
            with tc.tile_pool(name="gn_ps", bufs=1, space="PSUM") as gn_ps, \
                 tc.tile_pool(name="gn_wk", bufs=1) as gn_wk:
                wt = gn_wk.tile([128, 2, 3 * C], f32, tag="wt")
                for t in range(2):
                    nc.sync.dma_start(out=wt[:, t, :], in_=wqkvT_d[t * 128 : (t + 1) * 128, :])
                pt = gn_wk.tile([128, 2, C], f32, tag="pt")
                for t in range(2):
                    nc.sync.dma_start(out=pt[:, t, :], in_=wprojT_d[t * 128 : (t + 1) * 128, :])
                stats = gn_wk.tile([128, 2, 8, 6], f32, tag="stats")
                for t in range(2):
                    for j in range(8):
                        nc.vector.bn_stats(
                            out=stats[:, t, j, :],
                            in_=xs[:, t, j * 512 : (j + 1) * 512],
                        )
                mv = gn_wk.tile([128, 2, 2], f32, tag="mv")
                for t in range(2):
                    nc.vector.bn_aggr(out=mv[:, t, :], in_=stats[:, t, :, :])
                # per-channel {mean, var, mean^2}
                pcs = gn_wk.tile([128, 2, 3], f32, tag="pcs")
                for t in range(2):
                    nc.vector.tensor_copy(out=pcs[:, t, 0:2], in_=mv[:, t, :])
                    nc.vector.tensor_mul(
                        out=pcs[:, t, 2:3], in0=mv[:, t, 0:1], in1=mv[:, t, 0:1]
                    )
                gsum = gn_ps.tile([GROUPS, 3], f32, tag="gsum")
                for t in range(2):
                    nc.tensor.matmul(
                        gsum, ig[:, t, :], pcs[:, t, :], start=(t == 0), stop=(t == 1)
                    )
                gstats = gn_wk.tile([GROUPS, 3], f32, tag="gstats")
                nc.vector.tensor_scalar_mul(out=gstats, in0=gsum, scalar1=0.125)
                varg = gn_wk.tile([GROUPS, 1], f32, tag="varg")
                nc.vector.tensor_add(out=varg, in0=gstats[:, 1:2], in1=gstats[:, 2:3])
                mg2 = gn_wk.tile([GROUPS, 1], f32, tag="mg2")
                nc.vector.tensor_mul(out=mg2, in0=gstats[:, 0:1], in1=gstats[:, 0:1])
                nc.vector.tensor_tensor(
                    out=varg, in0=varg, in1=mg2, op=OP.subtract
                )
                # rstd = exp(-0.5 * ln(var + eps)) - stays in the exp table set
                eps_t = gn_wk.tile([GROUPS, 1], f32, tag="eps_t")
                nc.vector.memset(eps_t, EPS)
                lnv = gn_wk.tile([GROUPS, 1], f32, tag="lnv")
                nc.scalar.activation(out=lnv, in_=varg, func=AF.Ln, bias=eps_t)
                stats2 = gn_wk.tile([GROUPS, 2], f32, tag="stats2")
                nc.vector.tensor_copy(out=stats2[:, 0:1], in_=gstats[:, 0:1])
                nc.scalar.activation(
                    out=stats2[:, 1:2], in_=lnv, func=AF.Exp, scale=-0.5
                )
                cstat = gn_ps.tile([128, 2, 2], f32, tag="cstat")
                for t in range(2):
                    nc.tensor.matmul(
                        cstat[:, t, :],
                        igT[:, t * 128 : (t + 1) * 128],
                        stats2,
                        start=True,
                        stop=True,
                    )
                # A = rstd_c * gn_w ; B = gn_b - mean_c * A
                ab = singles.tile([128, 2, 2], f32, tag="ab")  # [..0]=A [..1]=B
                for t in range(2):
                    nc.vector.tensor_mul(
                        out=ab[:, t, 0:1], in0=cstat[:, t, 1:2], in1=gnw[:, t : t + 1]
                    )
                    nc.vector.tensor_mul(
                        out=ab[:, t, 1:2], in0=cstat[:, t, 0:1], in1=ab[:, t, 0:1]
                    )
                    nc.vector.tensor_tensor(
                        out=ab[:, t, 1:2],
                        in0=gnb[:, t : t + 1],
                        in1=ab[:, t, 1:2],
                        op=OP.subtract,
                    )
                # scaled qkv weights
                wts = singles.tile([128, 2, 3 * C], f32, tag="wts")
                for t in range(2):
                    nc.vector.tensor_scalar_mul(
                        out=R(wts[:, t, :]), in0=wt[:, t, :], scalar1=ab[:, t, 0:1]
                    )
                # bias_full = W @ B + qkv_b   (unscaled W)
                bf_ps = gn_ps.tile([128, 6], f32, tag="bf_ps")
                for m in range(6):
                    for t in range(2):
                        nc.tensor.matmul(
                            bf_ps[:, m : m + 1],
                            wt[:, t, m * 128 : (m + 1) * 128],
                            ab[:, t, 1:2],
                            start=(t == 0),
                            stop=(t == 1),
                        )
                biasf = singles.tile([128, 6], f32, tag="biasf")
                nc.vector.tensor_add(out=biasf, in0=bf_ps, in1=qkvb)
                # proj bias' = proj_b + P @ gamma, gamma = biasf v-part
                pb_ps = gn_ps.tile([128, 2], f32, tag="pb_ps")
                for m in range(2):
                    for t in range(2):
                        nc.tensor.matmul(
                            pb_ps[:, m : m + 1],
                            pt[:, t, m * 128 : (m + 1) * 128],
                            biasf[:, 4 + t : 5 + t],
                            start=(t == 0),
                            stop=(t == 1),
                        )
                pbf = singles.tile([128, 2], f32, tag="pbf")
                nc.vector.tensor_add(out=pbf, in0=pb_ps, in1=projb)

            # ---------------- qkv projections ----------------
            q_sb = singles.tile([128, 2, T], f32, tag="q_sb")
            k_sb = singles.tile([128, 2, L], f32, tag="k_sb")
            vt_sb = singles.tile([128, 32, 4, 65], f32, tag="vt_sb")
            ones_st = small.tile([128, 128], f32, tag="ones_st")
            nc.vector.memset(ones_st, 1.0)
            nc.vector.tensor_copy(
                out=R(vt_sb[:, :, :, 64:65]),
                in_=ones_st[:, :].rearrange("p (a b c) -> p a b c", a=32, b=4),
            )

            def emit_qkv(pair, qkv_ps):
                m = pair
                for n in range(4):
                    pp = qkv_ps.tile([128, 512], f32, tag="pp")
                    for t in range(2):
                        nc.tensor.matmul(
                            pp,
                            R(wts[:, t, m * 128 : (m + 1) * 128]),
                            R(xs[:, t, n * 512 : (n + 1) * 512]),
                            start=(t == 0),
                            stop=(t == 1),
                        )
                    nc.vector.tensor_scalar_add(
                        out=R(q_sb[:, m, n * 512 : (n + 1) * 512]),
                        in0=pp,
                        scalar1=biasf[:, m : m + 1],
                    )
                for n in range(8):
                    pp = qkv_ps.tile([128, 512], f32, tag="pp")
                    for t in range(2):
                        nc.tensor.matmul(
                            pp,
                            R(wts[:, t, 256 + m * 128 : 256 + (m + 1) * 128]),
                            R(xs[:, t, n * 512 : (n + 1) * 512]),
                            start=(t == 0),
                            stop=(t == 1),
                        )
                    nc.vector.tensor_scalar_add(
                        out=R(k_sb[:, m, n * 512 : (n + 1) * 512]),
                        in0=pp,
                        scalar1=biasf[:, 2 + m : 3 + m],
                    )
                if pair == 1:
                    return
                for sl in range(32):  # v^T all heads, no bias (folded into proj)
                    pp = qkv_ps.tile([128, 512], f32, tag="pp")
                    vv = pp[:, 0:256]
                    for t in range(2):
                        nc.tensor.matmul(
                            vv,
                            R(xs[:, t, sl * 128 : (sl + 1) * 128]),
                            R(wts[:, t, 512:768]),
                            start=(t == 0),
                            stop=(t == 1),
                        )
                    nc.vector.tensor_copy(
                        out=R(vt_sb[:, sl, :, 0:64]),
                        in_=vv.rearrange("p (h c) -> p h c", h=4),
                    )

            # ---------------- attention ----------------
            a_n = singles.tile([64, 4, T], f32, tag="a_n")

            with tc.tile_pool(name="qkv_ps", bufs=3, space="PSUM") as qkv_ps:
                emit_qkv(0, qkv_ps)
                emit_qkv(1, qkv_ps)
            with tc.tile_pool(name="scA", bufs=1, space="PSUM") as scA, \
                 tc.tile_pool(name="scB", bufs=1, space="PSUM") as scB, \
                 tc.tile_pool(name="av_ps", bufs=3, space="PSUM") as av_ps:
                for pair in range(2):
                    for tau in range(4):
                        t0, t1 = tau * 512, (tau + 1) * 512
                        av0 = av_ps.tile([65, 512], f32, tag="av")
                        av1 = av_ps.tile([65, 512], f32, tag="av")
                        av = [av0, av1]
                        n_slices = 64
                        groups = []
                        j = 0
                        size3 = True
                        while j < n_slices:
                            g = min(3 if size3 else 2, n_slices - j)
                            groups.append((j, g))
                            j += g
                            size3 = not size3

                        def emit_av(ew, gj, gsz):
                            for u in range(gsz):
                                sidx = (gj + u) // 2
                                hh = (gj + u) % 2
                                nc.tensor.matmul(
                                    av[hh],
                                    R(vt_sb[:, sidx, 2 * pair + hh, :]),
                                    R(ew[:, u * 512 : (u + 1) * 512]),
                                    start=(sidx == 0),
                                    stop=(sidx == 31),
                                )

                        pending = []
                        for gi, (gj, gsz) in enumerate(groups):
                            pool = scA if gsz == 3 else scB
                            st = pool.tile([128, gsz * 512], f32, tag="sc")
                            for u in range(gsz):
                                sidx = (gj + u) // 2
                                hh = (gj + u) % 2
                                lo, hi = hh * 64, hh * 64 + 64
                                nc.tensor.matmul(
                                    st[:, u * 512 : (u + 1) * 512],
                                    R(k_sb[lo:hi, pair, sidx * 128 : (sidx + 1) * 128]),
                                    R(q_sb[lo:hi, pair, t0:t1]),
                                    start=True,
                                    stop=True,
                                    tile_position=(lo, 0),
                                )
                            ew = ew_pool.tile([128, 3 * 512], f32, tag="ew")
                            nc.scalar.activation(
                                out=R(ew[:, 0 : gsz * 512]),
                                in_=st,
                                func=AF.Exp,
                                scale=0.125,
                            )
                            pending.append((ew, gj, gsz))
                            if len(pending) > 2:
                                emit_av(*pending.pop(0))
                        for args in pending:
                            emit_av(*args)

                        for hh in range(2):
                            den_s = small.tile([65, 512], f32, tag="den_s")
                            nc.vector.tensor_copy(
                                out=den_s[64:65, :], in_=av[hh][64:65, :]
                            )
                            den_d = dram.tile([1, 512], f32, tag="den_d")
                            nc.sync.dma_start(out=den_d, in_=den_s[64:65, :])
                            den_b = small.tile([64, 512], f32, tag="den_b")
                            nc.sync.dma_start(
                                out=den_b, in_=den_d[:, :].to_broadcast((64, 512))
                            )
                            rb = small.tile([64, 512], f32, tag="rb")
                            nc.vector.reciprocal_approx_fast(out=rb, in_=den_b)
                            nc.vector.tensor_mul(
                                out=R(a_n[:, 2 * pair + hh, t0:t1]),
                                in0=av[hh][0:64, :],
                                in1=rb,
                            )

            # ---------------- proj + residual ----------------
            with tc.tile_pool(name="pj_ps", bufs=4, space="PSUM") as pj_ps:
                for m in range(2):
                    for n in range(4):
                        pp = pj_ps.tile([128, 512], f32, tag="pj")
                        for h in range(4):
                            nc.tensor.matmul(
                                pp,
                                R(pt4[:, h, m * 128 : (m + 1) * 128]),
                                R(a_n[:, h, n * 512 : (n + 1) * 512]),
                                start=(h == 0),
                                stop=(h == 3),
                            )
                        ot = outp.tile([128, 512], f32, tag="ot")
                        nc.vector.tensor_scalar_add(
                            out=ot, in0=pp, scalar1=pbf[:, m : m + 1]
                        )
                        nc.vector.tensor_add(
                            out=ot, in0=ot, in1=xs[:, m, n * 512 : (n + 1) * 512]
                        )
                        nc.sync.dma_start(
                            out=out_d[m * 128 : (m + 1) * 128, n * 512 : (n + 1) * 512],
                            in_=ot,
                        )

    nc.finalize()
    return nc


def _get_nc():
    if "nc" not in _CACHE:
        _CACHE["nc"] = _build_bass()
    return _CACHE["nc"]


def _prepare_in_maps(x, gn_w, gn_b, qkv_w, qkv_b, proj_w, proj_b):
    x = np.asarray(x, np.float32)
    gn_w = np.asarray(gn_w, np.float32)
    gn_b = np.asarray(gn_b, np.float32)
    qkv_w = np.asarray(qkv_w, np.float32)
    qkv_b = np.asarray(qkv_b, np.float32)
    proj_w = np.asarray(proj_w, np.float32)
    proj_b = np.asarray(proj_b, np.float32)

    B, Cx, H, W = x.shape
    xf = x.reshape(B, Cx, H * W)

    # QKVAttentionLegacy: head h owns qkv rows [h*192, (h+1)*192) as q/k/v
    # blocks of 64. Permute to [q by head | k by head | v by head].
    perm = np.concatenate(
        [
            np.arange(h * 192 + j * 64, h * 192 + (j + 1) * 64)
            for j in range(3)
            for h in range(NUM_HEADS)
        ]
    )
    wqkvT = np.ascontiguousarray(qkv_w[perm].T)
    qkvb_p = np.ascontiguousarray(qkv_b[perm])
    wprojT = np.ascontiguousarray(proj_w.T)

    shared = {
        "wqkvT": wqkvT,
        "wprojT": wprojT,
        "qkvb": qkvb_p,
        "gnw": gn_w,
        "gnb": gn_b,
        "projb": proj_b,
    }
    in_maps = []
    for core in range(8):
        b, half = core // 2, core % 2
        if half == 0:
            xp = xf[b]
        else:
            xp = np.concatenate([xf[b][:, T:], xf[b][:, :T]], axis=1)
        in_maps.append({"xp": np.ascontiguousarray(xp), **shared})

    return in_maps, (B, Cx, H, W)


def _assemble(results, shape):
    B, Cx, H, W = shape
    out = np.empty((B, Cx, H * W), np.float32)
    for core in range(8):
        b, half = core // 2, core % 2
        out[b][:, half * T : (half + 1) * T] = results[core]["out"]
    return out.reshape(B, Cx, H, W)


def kernel(x, gn_w, gn_b, qkv_w, qkv_b, proj_w, proj_b):
    from concourse.bass_utils import run_bass_kernel_spmd

    in_maps, shape = _prepare_in_maps(x, gn_w, gn_b, qkv_w, qkv_b, proj_w, proj_b)
    nc = _get_nc()
    res = run_bass_kernel_spmd(nc, in_maps, core_ids=list(range(8)))
    _CACHE["last_results"] = res
    return _assemble(res.results, shape)

